# revision 20
# baseline (speedup 1.0000x reference)
"""Trainium2 Bass kernel for nn_AttentionTSSA (B=8, N=8192, C=512, H=8).

Sharding: data-parallel over batch B across the 8 NeuronCores (1 batch each,
no collectives).  All 16-bit tensors are fp16 (same engine speeds as bf16,
8x the mantissa - total rel err ~7e-4).  Per core:

  phase 1:  wT[c, n] = Wqkv @ x^T (fp16 GEMM, 512-token chunks; x DMA
            batched 4 chunks per transfer - dma_start has ~3us fixed latency
            on this part, so fewer/bigger transfers win).  Each PSUM
            bank is evacuated twice: DVE copies to fp16 wt (kept resident),
            ScalarE squares to fp16 w2 with accum_out giving per-channel
            norm^2 partials for free.
  finalize: inv[c] = 1/max(norm^2, eps); amat[ci] [128,16] fp16 packs
            inv*temp (cols 0:8) and ones (cols 8:16).
  window:   two 8-chunk "storms" of psB matmuls (w2-tile stationary, amat
            moving) land logits|r for all tokens in 2 PSUM banks; the
            DVE/ACT softmax chain then runs batched 4 chunks at a time
            (with temp==ones the mask multiply alone reproduces the
            reference masked softmax - uniform 1/8).  S[h]=sum Pi and
            PR[h]=sum Pi*r come from 2 ones-stationary matmuls per storm
            (ap=256) instead of a small-matmul storm.  Pi transposes to
            head-major pitc_all[8, N] per 2-chunk pair (PE transpose via
            identity; evacuations alternate ACT/DVE).  Then attn scalars:
            dots/attn from spr; -attn[h] expanded per channel partition via
            tiny ind8 matmuls (wex[ci]).
  phase 3:  per chunk, psE[c, n] = Pi[h(c), n] by a PE indicator matmul
            (ind8H stationary is constant per ci: ldweights fully hidden;
            a partition-broadcast DMA measures ~8us/transfer - avoid), then
            wts = wt * (-attn[h(c)]) * Pi in one scalar_tensor_tensor; the
            output GEMM outT = WoutT^T @ wts with bias fused into the PSUM
            evacuation; per-chunk output DMA (4KB contiguous lines in the
            blocked host layout) on the ACT HWDGE queue so it never queues
            behind phase-1 x loads.  Chunk 0's psE is emitted before the attn
            scalar chain so the output GEMM starts right after wex lands.

Timing note: unrolled multi-rep NEFFs are instruction-fetch bound on this
part (~4x slowdown at 33 reps); test.py measures with an on-device tc.For_i
hardware loop instead.

Host side transposes x per batch and un-transposes/upcasts the outputs.
"""

import numpy as np

B, N, C, H = 8, 8192, 512, 8
D = C // H          # 64
CT = C // 128       # 4 channel tiles
NCH = N // 512      # 16 chunks of 512 tokens
TPC = 4             # token tiles per chunk
NT = N // 128       # 64 token tiles

_CACHE = {}


def _build_bass(reps=1, debug=False, phases=(1, 2, 3), hwloop=0, p3skip=(),
                p1mode='', body=1):
    import concourse.bacc as bacc
    import concourse.bass as bass
    import concourse.mybir as mybir
    import concourse.tile as tile

    f32 = mybir.dt.float32
    bf16 = mybir.dt.float16
    Alu = mybir.AluOpType
    Act = mybir.ActivationFunctionType

    nc = bacc.Bacc("TRN2", target_bir_lowering=False, debug=False, num_devices=B)

    xP = nc.dram_tensor("xP", [NCH * 128, CT * 512], bf16, kind="ExternalInput")
    wqkvT = nc.dram_tensor("wqkvT", [C, C], bf16, kind="ExternalInput")
    woutT = nc.dram_tensor("woutT", [C, C], bf16, kind="ExternalInput")
    boutT = nc.dram_tensor("boutT", [128, CT], f32, kind="ExternalInput")
    maskf = nc.dram_tensor("maskf", [128, NT], f32, kind="ExternalInput")
    tempP = nc.dram_tensor("tempP", [128, CT], f32, kind="ExternalInput")
    identB = nc.dram_tensor("identB", [128, 128], bf16, kind="ExternalInput")
    ind8F = nc.dram_tensor("ind8F", [96, C], bf16, kind="ExternalInput")
    outP = nc.dram_tensor("outP", [NCH * 128, CT * 512], bf16, kind="ExternalOutput")
    if debug:
        dbg_inv = nc.dram_tensor("dbg_inv", [128, CT], f32, kind="ExternalOutput")
        dbg_spr = nc.dram_tensor("dbg_spr", [1, 16], f32, kind="ExternalOutput")
        dbg_watn = nc.dram_tensor("dbg_watn", [1, H], f32, kind="ExternalOutput")
        dbg_wex = nc.dram_tensor("dbg_wex", [128, CT], f32, kind="ExternalOutput")
        dbg_pi = nc.dram_tensor("dbg_pi", [128, NCH * TPC * H], bf16, kind="ExternalOutput")
        dbg_wt0 = nc.dram_tensor("dbg_wt0", [128, 512], bf16, kind="ExternalOutput")
        dbg_w20 = nc.dram_tensor("dbg_w20", [128, 512], bf16, kind="ExternalOutput")

    DCH = 1 if "dma1" in p1mode else (2 if "dma2" in p1mode else 4)
    XBUF = 2 if DCH > 1 else 3

    with tile.TileContext(nc) as tc:
        with (
            tc.tile_pool(name="singles", bufs=1) as sing,
            tc.tile_pool(name="workB", bufs=XBUF) as workB,
            tc.tile_pool(name="wts", bufs=12) as wtsp,
            tc.tile_pool(name="oc", bufs=3) as ocp,
            tc.tile_pool(name="small", bufs=2) as workS,
            tc.tile_pool(name="soft", bufs=2) as softp,
            tc.tile_pool(name="psu", bufs=8, space="PSUM") as psu,
        ):
            # ---------------- constants / persistent tiles ----------------
            wq = [sing.tile([128, C], bf16, tag=f"wq{i}", name=f"wq{i}") for i in range(CT)]
            wo = [sing.tile([128, C], bf16, tag=f"wo{i}", name=f"wo{i}") for i in range(CT)]
            wt = [sing.tile([128, N], bf16, tag=f"wt{i}", name=f"wt{i}") for i in range(CT)]
            w2 = [sing.tile([128, N], bf16, tag=f"w2_{i}", name=f"w2_{i}") for i in range(CT)]
            for i in range(CT):
                nc.sync.dma_start(out=wq[i][:], in_=wqkvT[i * 128:(i + 1) * 128, :])
            bout_sb = sing.tile([128, CT], f32, tag="bout", name="bout")
            maskf_sb = sing.tile([128, NT], f32, tag="maskf", name="maskf")
            tempP_sb = sing.tile([128, CT], f32, tag="tempP", name="tempP")
            identB_sb = sing.tile([128, 128], bf16, tag="identB", name="identB")
            ind8H = sing.tile([96, C], bf16, tag="ind8H", name="ind8H")

            def preload_rest():
                # emitted mid-phase1 so the critical xt DMAs clear HWDGE first
                for i in range(CT):
                    nc.sync.dma_start(out=wo[i][:], in_=woutT[i * 128:(i + 1) * 128, :])
                nc.sync.dma_start(out=bout_sb[:], in_=boutT[:])
                nc.sync.dma_start(out=maskf_sb[:], in_=maskf[:])
                nc.sync.dma_start(out=tempP_sb[:], in_=tempP[:])
                nc.sync.dma_start(out=identB_sb[:], in_=identB[:])
                nc.sync.dma_start(out=ind8H[:], in_=ind8F[:])

            ones1f = sing.tile([128, 1], f32, tag="ones1f", name="ones1f")
            nc.vector.memset(ones1f[:], 1.0)
            ones1b = sing.tile([128, 1], bf16, tag="ones1b", name="ones1b")
            nc.vector.tensor_copy(ones1b[:], ones1f[:])

            nsq = [sing.tile([128, NCH], f32, tag=f"nsq{i}", name=f"nsq{i}") for i in range(CT)]
            pi_all = sing.tile([128, NCH, TPC, H], bf16, tag="pi_all", name="pi_all")
            pirt_all = sing.tile([128, NCH, TPC, H], bf16, tag="pirt_all", name="pirt_all")
            pitc_all = sing.tile([96, 3 * 1024], bf16, tag="pitc_all", name="pitc_all")
            amat = [sing.tile([128, 16], bf16, tag=f"amat{i}", name=f"amat{i}") for i in range(CT)]
            inv = [sing.tile([128, 1], f32, tag=f"inv{i}", name=f"inv{i}") for i in range(CT)]
            wex = [sing.tile([128, 1], f32, tag=f"wex{i}", name=f"wex{i}") for i in range(CT)]
            spr = sing.tile([1, 16], f32, tag="spr", name="spr")
            sprg = [sing.tile([1, 2, H], f32, tag=f"sprg{g}", name=f"sprg{g}") for g in range(2)]
            watn = sing.tile([1, H], f32, tag="watn", name="watn")
            watnT = sing.tile([H, 1], bf16, tag="watnT", name="watnT")

            if 3 in phases and 2 not in phases:
                nc.vector.memset(pitc_all[:], 0.1)
                nc.vector.memset(pi_all[:], 0.1)
                for ci in range(CT):
                    nc.vector.memset(wex[ci][:], -0.5)

            for ci in range(CT):
                nc.vector.memset(amat[ci][:], 0.0)
                nc.vector.memset(amat[ci][0:64, 8 + 2 * ci:8 + 2 * ci + 1], 1.0)
                nc.vector.memset(amat[ci][64:128, 8 + 2 * ci + 1:8 + 2 * ci + 2], 1.0)

            xtb_static = [None]
            if "nodma" in p1mode:
                xtb_static[0] = sing.tile([128, CT, 512], bf16, tag="xtbs", name="xtbs")
                nc.vector.memset(xtb_static[0][:], 0.01)

            def phase1():
                xtb_cur = [None]
                for k in range(NCH):
                    if "nodma" in p1mode:
                        xtb = xtb_static[0]
                        xts = [xtb[:, ci, :] for ci in range(CT)]
                    else:
                        j = k % DCH
                        if j == 0:
                            xtb_cur[0] = workB.tile(
                                [128, DCH, CT * 512], bf16, tag="xtb", name="xtb")
                            xs = xP[k * 128:(k + DCH) * 128, :]
                            src = bass.AP(
                                tensor=xs.tensor, offset=xs.offset,
                                ap=[[xs.ap[0][0], 128], [128 * CT * 512, DCH],
                                    [1, CT * 512]])
                            eng = nc.scalar if ("dq" in p1mode and (k // DCH) % 2) else nc.sync
                            eng.dma_start(out=xtb_cur[0][:], in_=src)
                        xtb = xtb_cur[0]
                        xts = [xtb[:, j, ci * 512:(ci + 1) * 512] for ci in range(CT)]
                    # one PSUM bank at a time: evacuation of group co overlaps
                    # the matmuls of group co+1
                    for co in range(CT):
                        psA = psu.tile([128, 512], f32, tag="u", name="psA")
                        for ci in range(CT):
                            nc.tensor.matmul(
                                psA[:], wq[ci][:, co * 128:(co + 1) * 128],
                                xts[ci],
                                start=(ci == 0), stop=(ci == CT - 1),
                            )
                        # DVE: evacuate w to resident fp16 wt
                        nc.vector.tensor_copy(
                            wt[co][:, k * 512:(k + 1) * 512], psA[:])
                        # ScalarE: square to fp16 w2; accum_out = per-channel
                        # sum of squares for this chunk (norm^2 partials)
                        nc.scalar.activation(
                            out=w2[co][:, k * 512:(k + 1) * 512], in_=psA[:],
                            func=Act.Square, accum_out=nsq[co][:, k:k + 1])

            def norm_finalize():
                # amat's ones/zero pattern is static (set once at build);
                # per rep only the inv*temp values in cols 2ci/2ci+1 change
                for ci in range(CT):
                    nsqt = workS.tile([128, 1], f32, tag="nsqt", name="nsqt")
                    nc.vector.reduce_sum(nsqt[:], nsq[ci][:], axis=mybir.AxisListType.X)
                    nc.vector.tensor_scalar_max(nsqt[:], nsqt[:], 1e-24)
                    nc.vector.reciprocal(inv[ci][:], nsqt[:])
                    nc.vector.tensor_scalar_mul(
                        inv[ci][:], inv[ci][:], tempP_sb[:, ci:ci + 1])
                    nc.vector.tensor_copy(
                        amat[ci][0:64, 2 * ci:2 * ci + 1], inv[ci][0:64, :])
                    nc.vector.tensor_copy(
                        amat[ci][64:128, 2 * ci + 1:2 * ci + 2], inv[ci][64:128, :])

            GCH = 8       # chunks per psB storm (one PSUM bank each)
            SCH = 4       # chunks per softmax-chain batch

            def chain(c0):
                # batched softmax for chunks c0..c0+SCH (psum bank g=c0//GCH)
                ps = storms[c0 // GCH]
                sl = ps[:, (c0 % GCH):(c0 % GCH) + SCH, :, :]
                lg = softp.tile([128, SCH, TPC, H], f32, tag="lg", name="lg")
                mf = maskf_sb[:, c0 * TPC:(c0 + SCH) * TPC]
                mfb = bass.AP(tensor=mf.tensor, offset=mf.offset,
                              ap=[mf.ap[0], mf.ap[1], [0, H]])
                # logits = sum_ws*temp*mask: with temp==ones a masked token
                # gets all-zero logits -> exactly the reference's uniform
                # 1/8 softmax, so no mask bias is needed at all.
                nc.vector.tensor_mul(
                    lg[:].rearrange("p a b h -> p (a b) h"), sl[:, :, :, 0:H],
                    mfb)
                elg = softp.tile([128, SCH, TPC, H], bf16, tag="elg", name="elg")
                nc.scalar.activation(out=elg[:], in_=lg[:], func=Act.Exp)
                erec = softp.tile([128, SCH, TPC], f32, tag="erec", name="erec")
                nc.vector.reduce_sum(erec[:], elg[:], axis=mybir.AxisListType.X)
                nc.vector.reciprocal(erec[:], erec[:])
                er = erec[:]
                erb = bass.AP(tensor=er.tensor, offset=er.offset,
                              ap=[er.ap[0], er.ap[1], er.ap[2], [0, H]])
                nc.vector.tensor_mul(pi_all[:, c0:c0 + SCH, :, :], elg[:], erb)
                nc.vector.tensor_mul(
                    pirt_all[:, c0:c0 + SCH, :, :],
                    pi_all[:, c0:c0 + SCH, :, :], sl[:, :, :, 8:16])

            storms = {}

            def storm(g):
                # all psB matmuls for chunks g*8..g*8+8 into one PSUM bank
                ps = psu.tile([128, GCH, TPC, 16], f32, tag="u", name="psBst")
                storms[g] = ps
                for kk in range(GCH):
                    k = g * GCH + kk
                    for ti in range(TPC):
                        t = k * TPC + ti
                        for ci in range(CT):
                            nc.tensor.matmul(
                                ps[:, kk, ti, :],
                                w2[ci][:, t * 128:(t + 1) * 128],
                                amat[ci][:],
                                start=(ci == 0), stop=(ci == CT - 1))

            def psS_group(g):
                # S[h] = sum_n Pi, PR[h] = sum_n Pi*r for chunks g*8..g*8+8,
                # via two ones-stationary matmuls + one DVE reduce
                psSg = psu.tile([1, 2, GCH * TPC * H], f32, tag="u", name="psS")
                nc.tensor.matmul(
                    psSg[0:1, 0, :], ones1b[:, 0:1],
                    pi_all[:, g * GCH:(g + 1) * GCH, :, :], start=True, stop=True)
                nc.tensor.matmul(
                    psSg[0:1, 1, :], ones1b[:, 0:1],
                    pirt_all[:, g * GCH:(g + 1) * GCH, :, :], start=True, stop=True)
                nc.vector.reduce_sum(
                    sprg[g][:],
                    psSg[:].rearrange("p a (k h) -> p a h k", k=GCH * TPC, h=H),
                    axis=mybir.AxisListType.X)

            def stageA2(p):
                # Pi transposes for chunk pair p into one psT2 bank; pitc_all
                # written per pair (head-major, fp16, unscaled); evacuation
                # engine alternates so neither ACT nor DVE becomes the gate
                psT2 = psu.tile([H, 2, TPC, 128], bf16, tag="u", name="psT2")
                for jh in range(2):
                    j = 2 * p + jh
                    for ti in range(TPC):
                        nc.tensor.transpose(
                            psT2[:, jh, ti, :], pi_all[:, j, ti, :], identB_sb[:])
                b = 32 * (p % 3)
                dst = pitc_all[b:b + 8,
                               (p // 3) * 1024:(p // 3) * 1024 + 1024]
                if p % 2 == 0:
                    nc.scalar.activation(
                        out=dst, in_=psT2[:].rearrange("p a b c -> p (a b c)"),
                        func=Act.Copy)
                else:
                    nc.vector.tensor_copy(
                        dst, psT2[:].rearrange("p a b c -> p (a b c)"))

            def phase2():
                storm(0)
                storm(1)
                chain(0)
                chain(4)
                chain(8)
                chain(12)
                psS_group(0)
                stageA2(0)
                stageA2(1)
                psS_group(1)
                for p in range(2, NCH // 2):
                    stageA2(p)

            def global_scalars():
                # spr[0,0:8] = S[h], spr[0,8:16] = PR[h]
                nc.vector.tensor_add(
                    spr[:].rearrange("p (a h) -> p a h", a=2),
                    sprg[0][:], sprg[1][:])
                srec = workS.tile([1, H], f32, tag="srec", name="srec")
                nc.vector.tensor_scalar_add(srec[:], spr[0:1, 0:H], 1e-8)
                nc.vector.reciprocal(srec[:], srec[:])
                dots = workS.tile([1, H], f32, tag="dots", name="dots")
                nc.vector.tensor_mul(dots[:], spr[0:1, H:2 * H], srec[:])
                nc.vector.tensor_scalar_add(dots[:], dots[:], 1.0)
                nc.vector.reciprocal(watn[:], dots[:])
                nc.vector.tensor_scalar_mul(watn[:], watn[:], -1.0)
                psW = psu.tile([H, 1], f32, tag="u", name="psW")
                nc.tensor.matmul(psW[:], watn[:], ones1f[0:1, 0:1], is_transpose=True)
                nc.scalar.activation(out=watnT[:], in_=psW[:], func=Act.Copy)
                # wex[ci][p] = -attn[h(p, ci)]: expand across channel partitions
                for ci in range(CT):
                    psWE = psu.tile([128, 1], f32, tag="u", name="psWE")
                    nc.tensor.matmul(
                        psWE[:], ind8H[0:8, ci * 128:(ci + 1) * 128], watnT[:],
                        start=True, stop=True)
                    nc.scalar.activation(out=wex[ci][:], in_=psWE[:], func=Act.Copy)

            wtss = {}

            def stageC(k):
                wts = wtss.pop(k)
                oc = ocp.tile([128, CT, 512], bf16, tag="outc", name="outc")
                for oj in range(CT):
                    psC = psu.tile([128, 512], f32, tag="u", name="psC")
                    for ci in range(CT):
                        nc.tensor.matmul(
                            psC[:], wo[ci][:, oj * 128:(oj + 1) * 128],
                            wts[ci][:],
                            start=(ci == 0), stop=(ci == CT - 1))
                    nc.scalar.activation(
                        out=oc[:, oj, :], in_=psC[:], func=Act.Identity,
                        bias=bout_sb[:, oj:oj + 1], scale=1.0)
                # per-chunk output DMA (4KB contiguous lines in the blocked
                # host layout) on the ACT HWDGE queue
                if "odma" not in p3skip:
                    nc.scalar.dma_start(
                        out=outP[k * 128:(k + 1) * 128, :], in_=oc[:])

            pse_pend = {}

            def stageB_pse(k):
                if "nopse" in p3skip or "nowts" in p3skip:
                    return
                pses = []
                p = k // 2
                b = 32 * (p % 3)
                c0 = (p // 3) * 1024 + (k % 2) * 512
                pit = pitc_all[b:b + 8, c0:c0 + 512]
                for ci in range(CT):
                    psE = psu.tile([128, 512], f32, tag="u", name="psE")
                    nc.tensor.matmul(
                        psE[:], ind8H[b:b + 8, ci * 128:(ci + 1) * 128], pit,
                        start=True, stop=True)
                    pses.append(psE)
                pse_pend[k] = pses

            def stageB_stt(k):
                if "nowts" in p3skip:
                    wtss[k] = [wt[ci][:, k * 512:(k + 1) * 512]
                               for ci in range(CT)]
                    return
                wts = []
                for ci in range(CT):
                    w = wtsp.tile([128, 512], bf16, tag="wts", name="wts")
                    if "nopse" in p3skip:
                        nc.vector.tensor_scalar_mul(
                            w[:], wt[ci][:, k * 512:(k + 1) * 512], wex[ci][:])
                    else:
                        nc.vector.scalar_tensor_tensor(
                            out=w[:], in0=wt[ci][:, k * 512:(k + 1) * 512],
                            scalar=wex[ci][:], in1=pse_pend[k][ci][:],
                            op0=Alu.mult, op1=Alu.mult)
                    wts.append(w)
                if "nopse" not in p3skip:
                    del pse_pend[k]
                wtss[k] = wts

            def phase3():
                if 0 not in pse_pend and "nopse" not in p3skip and "nowts" not in p3skip:
                    stageB_pse(0)
                stageB_stt(0)
                for k in range(NCH):
                    if k + 1 < NCH:
                        stageB_pse(k + 1)
                        stageB_stt(k + 1)
                    stageC(k)

            def rep_body(first):
                if 1 in phases:
                    phase1()
                if first:
                    preload_rest()
                if 1 in phases:
                    norm_finalize()
                if 2 in phases:
                    phase2()
                    # chunk 0's Pi expansion overlaps the attn scalar chain:
                    # psE needs only pitc, while the STT waits for wex
                    if 3 in phases:
                        stageB_pse(0)
                    global_scalars()
                if 3 in phases:
                    phase3()

            if hwloop > 1:
                rep_body(True)
                with tc.For_i(0, (hwloop - 1) // body, 1):
                    for _ in range(body):
                        rep_body(False)
            else:
                for _rep in range(reps):
                    rep_body(_rep == 0)
            if debug:
                for ci in range(CT):
                    nc.sync.dma_start(out=dbg_inv[:, ci:ci + 1], in_=inv[ci][:])
                    nc.sync.dma_start(out=dbg_wex[:, ci:ci + 1], in_=wex[ci][:])
                nc.sync.dma_start(out=dbg_spr[:], in_=spr[:])
                nc.sync.dma_start(out=dbg_watn[:], in_=watn[:])
                nc.sync.dma_start(out=dbg_pi[:], in_=pi_all[:].rearrange("p a b c -> p (a b c)"))
                nc.sync.dma_start(out=dbg_wt0[:], in_=wt[0][:, 0:512])
                nc.sync.dma_start(out=dbg_w20[:], in_=w2[0][:, 0:512])

    nc.compile()
    return nc


def _prep_inputs(x, token_mask, Wqkv, temp, Wout, bout):
    f = np.float32
    bf = np.float16
    temp = np.asarray(temp, dtype=f)
    wqkvT = np.ascontiguousarray(np.asarray(Wqkv, f).T.astype(bf))
    woutT = np.ascontiguousarray(np.asarray(Wout, f).T.astype(bf))
    boutT = np.ascontiguousarray(np.asarray(bout, f).reshape(CT, 128).T)
    identB = np.eye(128, dtype=bf)
    ind8one = (np.arange(C) // D == np.arange(H)[:, None]).astype(bf)
    ind8F = np.zeros((96, C), bf)
    for base in (0, 32, 64):
        ind8F[base:base + H] = ind8one
    # tempP[p, ci] = temp[2ci + (p>=64)]
    tempP = np.empty((128, CT), f)
    for ci in range(CT):
        tempP[0:64, ci] = temp[2 * ci, 0]
        tempP[64:128, ci] = temp[2 * ci + 1, 0]
    in_maps = []
    for b in range(B):
        m = np.asarray(token_mask[b], f)          # [N]
        mt = m.reshape(NT, 128).T.copy()          # [128, NT]
        xTb = np.asarray(x[b], f).T.astype(bf)          # [C, N]
        xPb = np.ascontiguousarray(
            xTb.reshape(CT, 128, NCH, 512).transpose(2, 1, 0, 3)
        ).reshape(NCH * 128, CT * 512)
        in_maps.append({
            "xP": xPb,
            "wqkvT": wqkvT,
            "woutT": woutT,
            "boutT": boutT,
            "maskf": mt,
            "tempP": tempP,
            "identB": identB,
            "ind8F": ind8F,
        })
    return in_maps


def kernel(**inputs):
    from concourse.bass_utils import run_bass_kernel_spmd

    if "nc" not in _CACHE:
        _CACHE["nc"] = _build_bass()
    nc = _CACHE["nc"]
    in_maps = _prep_inputs(**inputs)
    try:
        res = run_bass_kernel_spmd(nc, in_maps, core_ids=list(range(B)))
    except Exception:
        # transient device/tunnel hiccup: retry once
        import time as _t
        _t.sleep(2.0)
        res = run_bass_kernel_spmd(nc, in_maps, core_ids=list(range(B)))
    out = np.empty((B, N, C), np.float32)
    for b in range(B):
        op = np.asarray(res.results[b]["outP"], dtype=np.float32)
        outT = op.reshape(NCH, 128, CT, 512).transpose(2, 1, 0, 3).reshape(C, N)
        out[b] = outT.T
    return out
